# revision 37
# baseline (speedup 1.0000x reference)
"""Trainium2 Bass kernel for nn_Encoder_82411832475895.

16-step GRU encoder with per-step weights + per-step Predict heads, then
mu/log_var projections.  Data-parallel over batch across 8 NeuronCores
(512 rows/core); activations kept transposed on-chip ([feature, batch])
so batch is the matmul free dim and weights are PE-stationary.

Self-contained: takes the full unsharded inputs, shards/packs on host,
compiles+runs the Bass kernel SPMD on cores 0-7, gathers to full output.
"""

import math

import numpy as np

B, X, H, Z, L = 4096, 512, 1024, 128, 16
NCORES = 8
BL = B // NCORES  # 512 batch rows per core
P = 128
HK = H // P  # 8
XK = X // P  # 4
GK = 3 * H // P  # 24 gate chunks: 0..7 r, 8..15 u(z), 16..23 n
STD = float(math.exp(0.5 * 0.005))

# "f32" (exact, 4 cyc/row) or "bf16"
COMPUTE_DT = "bf16"


def _pack_host(inputs):
    """Shard + lay out inputs for the on-chip program."""
    import ml_dtypes

    f32 = np.float32
    wnp = ml_dtypes.bfloat16 if COMPUTE_DT == "bf16" else f32

    g = {k: np.asarray(v, f32) for k, v in inputs.items()}

    x = g["x"]  # [B, X]
    eps_steps = g["eps_steps"]  # [L, B, H]
    eps_final = g["eps_final"]  # [B, Z]

    # lhsT chunk layout [.., m, i(K), k, j(M)]: w[m, i, k, j] = W[m*P+j, k*P+i]
    wih = np.ascontiguousarray(
        g["gru_Wih"][:, :, :X].reshape(L, GK, P, XK, P).transpose(0, 1, 4, 3, 2)
    ).astype(wnp)
    whh = np.ascontiguousarray(
        g["gru_Whh"].reshape(L, GK, P, HK, P).transpose(0, 1, 4, 3, 2)
    ).astype(wnp)
    w1 = np.ascontiguousarray(
        g["pr_W1"].reshape(L, HK, P, HK, P).transpose(0, 1, 4, 3, 2)
    ).astype(wnp)
    w2 = np.ascontiguousarray(
        g["pr_W2"][:, 0, :].reshape(L, HK, P).transpose(0, 2, 1)
    ).astype(wnp)  # [L, P(i), HK(k)]
    # wt: Wih t-column per gate chunk -> per-partition scalar [L, P, GK]
    wt = np.ascontiguousarray(
        g["gru_Wih"][:, :, X].reshape(L, GK, P).transpose(0, 2, 1)
    ).astype(f32)
    # gb: gate bias per chunk: bih+bhh for r/u, bih for n  [L, P, GK]
    gb = (g["gru_bih"] + g["gru_bhh"]).reshape(L, GK, P).copy()
    gb[:, 2 * HK :] = g["gru_bih"].reshape(L, GK, P)[:, 2 * HK :]
    gb = np.ascontiguousarray(gb.transpose(0, 2, 1)).astype(f32)
    # bhhn: recurrent bias of the n gate [L, P, HK]
    bhhn = np.ascontiguousarray(
        g["gru_bhh"][:, 2 * H :].reshape(L, HK, P).transpose(0, 2, 1)
    ).astype(f32)
    b1s = np.ascontiguousarray(
        g["pr_b1"].reshape(L, HK, P).transpose(0, 2, 1)
    ).astype(f32)
    fw1 = np.ascontiguousarray(
        g["fc_W1"].reshape(HK, P, XK, P).transpose(3, 0, 2, 1)
    ).astype(wnp)  # [P(i), HK(m), XK(k), P(j)]
    fb1 = np.ascontiguousarray(g["fc_b1"].reshape(HK, P).T).astype(f32)
    fw2 = np.ascontiguousarray(g["fc_W2"][0].reshape(HK, P).T).astype(wnp)
    muW = np.ascontiguousarray(
        g["mu_W"].T.reshape(HK, P, Z).transpose(1, 0, 2)
    ).astype(wnp)  # [P(i), HK(k), Z(j)]
    lvW = np.ascontiguousarray(
        g["lv_W"].T.reshape(HK, P, Z).transpose(1, 0, 2)
    ).astype(wnp)
    mub = np.ascontiguousarray(g["mu_b"].reshape(Z, 1)).astype(f32)
    lvb = np.ascontiguousarray(g["lv_b"].reshape(Z, 1)).astype(f32)

    pr_b2 = [float(v) for v in np.asarray(g["pr_b2"]).reshape(L)]
    fc_b2 = float(np.asarray(g["fc_b2"]).reshape(()))
    b2s = np.array([[fc_b2] + pr_b2], f32).astype(wnp)  # [1, L+1]

    shared = {
        "wih": wih, "whh": whh, "w1": w1, "w2": w2, "wt": wt, "gb": gb,
        "bhhn": bhhn, "b1s": b1s, "fw1": fw1, "fb1": fb1, "fw2": fw2,
        "muW": muW, "lvW": lvW, "mub": mub, "lvb": lvb, "b2s": b2s,
    }

    in_maps = []
    for c in range(NCORES):
        sl = slice(c * BL, (c + 1) * BL)
        xT = np.ascontiguousarray(
            x[sl].T.reshape(XK, P, BL).transpose(1, 0, 2)
        ).astype(wnp)  # [P(i), XK(k), BL]
        epsT = np.ascontiguousarray(
            eps_steps[:, sl, :].transpose(0, 2, 1).reshape(L, HK, P, BL)
            .transpose(0, 2, 1, 3)
        ).astype(f32)  # [L, P(i), HK(h), BL]
        epsf = np.ascontiguousarray(eps_final[sl].T).astype(f32)  # [Z, BL]
        in_maps.append({"xT": xT, "epsT": epsT, "epsf": epsf, **shared})

    return in_maps, pr_b2, fc_b2


def _build(nc):
    import concourse.bass as bass  # noqa: F401
    import concourse.tile as tile
    from concourse import mybir
    from concourse.alu_op_type import AluOpType as OP

    AF = mybir.ActivationFunctionType
    f32 = mybir.dt.float32
    WDT = mybir.dt.bfloat16 if COMPUTE_DT == "bf16" else f32

    # ---- DRAM tensors ----
    xT_d = nc.dram_tensor("xT", [P, XK, BL], WDT, kind="ExternalInput")
    epsT_d = nc.dram_tensor("epsT", [L, P, HK, BL], f32, kind="ExternalInput")
    epsf_d = nc.dram_tensor("epsf", [Z, BL], f32, kind="ExternalInput")
    wih_d = nc.dram_tensor("wih", [L, GK, P, XK, P], WDT, kind="ExternalInput")
    whh_d = nc.dram_tensor("whh", [L, GK, P, HK, P], WDT, kind="ExternalInput")
    w1_d = nc.dram_tensor("w1", [L, HK, P, HK, P], WDT, kind="ExternalInput")
    w2_d = nc.dram_tensor("w2", [L, P, HK], WDT, kind="ExternalInput")
    wt_d = nc.dram_tensor("wt", [L, P, GK], f32, kind="ExternalInput")
    gb_d = nc.dram_tensor("gb", [L, P, GK], f32, kind="ExternalInput")
    bhhn_d = nc.dram_tensor("bhhn", [L, P, HK], f32, kind="ExternalInput")
    b1s_d = nc.dram_tensor("b1s", [L, P, HK], f32, kind="ExternalInput")
    fw1_d = nc.dram_tensor("fw1", [P, HK, XK, P], WDT, kind="ExternalInput")
    fb1_d = nc.dram_tensor("fb1", [P, HK], f32, kind="ExternalInput")
    fw2_d = nc.dram_tensor("fw2", [P, HK], WDT, kind="ExternalInput")
    muW_d = nc.dram_tensor("muW", [P, HK, Z], WDT, kind="ExternalInput")
    lvW_d = nc.dram_tensor("lvW", [P, HK, Z], WDT, kind="ExternalInput")
    mub_d = nc.dram_tensor("mub", [Z, 1], f32, kind="ExternalInput")
    lvb_d = nc.dram_tensor("lvb", [Z, 1], f32, kind="ExternalInput")
    b2s_d = nc.dram_tensor("b2s", [1, L + 1], WDT, kind="ExternalInput")
    lat_o = nc.dram_tensor("lat_o", [Z, BL], f32, kind="ExternalOutput")
    mu_o = nc.dram_tensor("mu_o", [Z, BL], f32, kind="ExternalOutput")
    lv_o = nc.dram_tensor("lv_o", [Z, BL], f32, kind="ExternalOutput")
    t_o = nc.dram_tensor("t_o", [L, BL], f32, kind="ExternalOutput")

    with tile.TileContext(nc) as tc:
        with (
            tc.tile_pool(name="const", bufs=1) as cpool,
            tc.tile_pool(name="wts", bufs=1) as wpool,
            tc.tile_pool(name="act", bufs=1) as apool,
            tc.tile_pool(name="psum", bufs=8, space="PSUM") as pspool,
        ):
            # ---- resident constants ----
            x_sb = cpool.tile([P, XK, BL], WDT)
            for k in range(XK):
                nc.sync.dma_start(x_sb[:, k, :], xT_d.ap()[:, k, :])
            epsf_sb = cpool.tile([Z, BL], f32)
            nc.sync.dma_start(epsf_sb[:], epsf_d.ap())
            fb1_sb = cpool.tile([P, HK], f32)
            nc.sync.dma_start(fb1_sb[:], fb1_d.ap())
            fw2_sb = cpool.tile([P, HK], WDT)
            nc.sync.dma_start(fw2_sb[:], fw2_d.ap())
            muW_sb = cpool.tile([P, HK, Z], WDT)
            nc.sync.dma_start(muW_sb[:], muW_d.ap())
            lvW_sb = cpool.tile([P, HK, Z], WDT)
            nc.sync.dma_start(lvW_sb[:], lvW_d.ap())
            mub_sb = cpool.tile([Z, 1], f32)
            nc.sync.dma_start(mub_sb[:], mub_d.ap())
            lvb_sb = cpool.tile([Z, 1], f32)
            nc.sync.dma_start(lvb_sb[:], lvb_d.ap())
            fw1_sb = cpool.tile([P, HK, XK, P], WDT)
            for m in range(HK):
                nc.sync.dma_start(fw1_sb[:, m], fw1_d.ap()[:, m])
            b2_sb = cpool.tile([1, L + 1], WDT)
            nc.sync.dma_start(b2_sb[:], b2s_d.ap())
            ones_sb = cpool.tile([1, BL], WDT)
            nc.vector.memset(ones_sb[:], 1.0)
            ones128 = cpool.tile([1, P], WDT)
            nc.vector.memset(ones128[:], 1.0)

            # per-h z state tiles (separate tiles -> fine-grained deps)
            z_cur = []
            zb_cur = []
            for h in range(HK):
                zt = apool.tile([P, BL], f32, tag=f"z{h}", bufs=2, name=f"z0_{h}")
                nc.vector.memset(zt[:], 0.0)
                z_cur.append(zt)
                zbt = apool.tile([P, BL], WDT, tag=f"zb{h}", bufs=2, name=f"zb0_{h}")
                nc.vector.memset(zbt[:], 0.0)
                zb_cur.append(zbt)

            def predict(l, w1s, w2s, b1ss, zb, is_first_cell=False):
                """Two-wave W1/W2 head; returns the logit psum [1, BL]."""
                ps2 = pspool.tile([1, BL], f32, tag="ps", name=f"ps2_{l}")
                psp = {}
                for wave in (range(0, 4), range(4, 8)):
                    for m in wave:
                        psp[m] = pspool.tile([P, BL], f32, tag="ps", name=f"pp{l}_{m}")
                        for k in range(HK - 1):
                            nc.tensor.matmul(
                                psp[m][:], w1s(m, k), zb[k], start=(k == 0), stop=False
                            )
                    for m in wave:
                        nc.tensor.matmul(
                            psp[m][:], w1s(m, HK - 1), zb[HK - 1],
                            start=False, stop=True,
                        )
                    for m in wave:
                        h1 = apool.tile([P, BL], WDT, tag="h1", bufs=4, name=f"h1_{m}")
                        nc.scalar.activation(h1[:], psp[m][:], AF.Relu, bias=b1ss(m))
                        nc.tensor.matmul(
                            ps2[:], w2s(m), h1[:], start=(m == 0), stop=False
                        )
                bidx = 0 if is_first_cell else l + 1
                nc.tensor.matmul(
                    ps2[:], b2_sb[0:1, bidx : bidx + 1], ones_sb[:],
                    start=False, stop=True,
                )
                return ps2

            def head_t(l, ps2):
                """sigmoid -> t row DMA; is_ge -> tmp row (partition 0)."""
                row = 0 if l is None else l + 1
                t_row = apool.tile([1, BL], f32, tag="trow", bufs=2, name=f"tr{row}")
                nc.scalar.activation(t_row[:], ps2[:], AF.Sigmoid)
                nc.sync.dma_start(t_o.ap()[row : row + 1, :], t_row[:])
                tmp_row = apool.tile([1, BL], WDT, tag="tmp", bufs=2, name=f"tm{row}")
                nc.vector.tensor_scalar(tmp_row[:], ps2[:], 0.0, None, OP.is_ge)
                return tmp_row

            def bcast_tmp(tmp_row, row):
                """Broadcast tmp to all partitions: rank-1 matmul + copy.
                Emitted a little into the next step so the PE FIFO isn't
                head-of-line blocked waiting for is_ge."""
                ps_b = pspool.tile([P, BL], f32, tag="ps", name=f"psb{row}")
                nc.tensor.matmul(ps_b[:], ones128[:], tmp_row[:], start=True, stop=True)
                tmp_bc = apool.tile([P, BL], f32, tag="tbc", bufs=2, name=f"tb{row}")
                nc.scalar.copy(tmp_bc[:], ps_b[:])
                return tmp_bc

            # ---- firstcell (K=XK, written out explicitly) ----
            ps2 = pspool.tile([1, BL], f32, tag="ps", name="ps2_fc")
            for m in range(HK):
                psf = pspool.tile([P, BL], f32, tag="ps", name="ps_fc")
                for k in range(XK):
                    nc.tensor.matmul(
                        psf[:], fw1_sb[:, m, k, :], x_sb[:, k, :],
                        start=(k == 0), stop=(k == XK - 1),
                    )
                h1 = apool.tile([P, BL], WDT, tag="h1", bufs=4, name="h1_fc")
                nc.scalar.activation(h1[:], psf[:], AF.Relu, bias=fb1_sb[:, m : m + 1])
                nc.tensor.matmul(
                    ps2[:], fw2_sb[:, m : m + 1], h1[:], start=(m == 0), stop=False
                )
            nc.tensor.matmul(
                ps2[:], b2_sb[0:1, 0:1], ones_sb[:], start=False, stop=True
            )
            tmp_pending = (head_t(None, ps2), 0)
            tmp_bc = None

            # ---- L GRU steps ----
            for l in range(L):
                first, last = l == 0, l == L - 1
                wt_sb = wpool.tile([P, GK], f32, tag="wt", bufs=2, name=f"wt{l}")
                nc.sync.dma_start(wt_sb[:], wt_d.ap()[l])
                gb_sb = wpool.tile([P, GK], f32, tag="gb", bufs=2, name=f"gb{l}")
                nc.sync.dma_start(gb_sb[:], gb_d.ap()[l])
                bhhn_sb = wpool.tile([P, HK], f32, tag="bhhn", bufs=2, name=f"bh{l}")
                nc.sync.dma_start(bhhn_sb[:], bhhn_d.ap()[l])
                if not last:
                    b1s_sb = wpool.tile([P, HK], f32, tag="b1s", bufs=2, name=f"b1{l}")
                    nc.sync.dma_start(b1s_sb[:], b1s_d.ap()[l])
                    w2_sb = wpool.tile([P, HK], WDT, tag="w2", bufs=2, name=f"w2{l}")
                    nc.sync.dma_start(w2_sb[:], w2_d.ap()[l])

                z_new, zb_new = [], []

                def gate_mms(m, with_z, l=l, first=first):
                    """PSUM accumulation of gate chunk m (x part [+ z part])."""
                    wih_sb = wpool.tile(
                        [P, XK, P], WDT, tag="wih", bufs=12, name=f"wi{l}_{m}"
                    )
                    nc.sync.dma_start(wih_sb[:], wih_d.ap()[l, m])
                    ps = pspool.tile([P, BL], f32, tag="ps", name=f"ps{l}_{m}")
                    use_z = with_z and not first
                    for k in range(XK):
                        nc.tensor.matmul(
                            ps[:], wih_sb[:, k, :], x_sb[:, k, :],
                            start=(k == 0), stop=(k == XK - 1 and not use_z),
                        )
                    if use_z:
                        whh_sb = wpool.tile(
                            [P, HK, P], WDT, tag="whh", bufs=10, name=f"wh{l}_{m}"
                        )
                        nc.sync.dma_start(whh_sb[:], whh_d.ap()[l, m])
                        for k in range(HK):
                            nc.tensor.matmul(
                                ps[:], whh_sb[:, k, :], zb_cur[k][:],
                                start=False, stop=(k == HK - 1),
                            )
                    return ps

                for h in range(HK):
                    ps_r = gate_mms(h, True)
                    if h == 0:
                        tmp_bc = bcast_tmp(*tmp_pending)
                    pre_r = apool.tile([P, BL], f32, tag="pre", bufs=3, name=f"pr{h}")
                    nc.vector.scalar_tensor_tensor(
                        pre_r[:], tmp_bc[:], wt_sb[:, h : h + 1], ps_r[:],
                        OP.mult, OP.add,
                    )
                    r_t = apool.tile([P, BL], f32, tag="r", bufs=2, name=f"r{l}_{h}")
                    nc.scalar.activation(
                        r_t[:], pre_r[:], AF.Sigmoid, bias=gb_sb[:, h : h + 1]
                    )

                    ps_u = gate_mms(HK + h, True)
                    pre_u = apool.tile([P, BL], f32, tag="pre", bufs=3, name=f"pu{h}")
                    nc.vector.scalar_tensor_tensor(
                        pre_u[:], tmp_bc[:], wt_sb[:, HK + h : HK + h + 1], ps_u[:],
                        OP.mult, OP.add,
                    )
                    u_t = apool.tile([P, BL], f32, tag="u", bufs=2, name=f"u{l}_{h}")
                    nc.scalar.activation(
                        u_t[:], pre_u[:], AF.Sigmoid, bias=gb_sb[:, HK + h : HK + h + 1]
                    )

                    ps_gx = gate_mms(2 * HK + h, False)
                    pre_n = apool.tile([P, BL], f32, tag="pre", bufs=3, name=f"pn{h}")
                    nc.vector.scalar_tensor_tensor(
                        pre_n[:], tmp_bc[:], wt_sb[:, 2 * HK + h : 2 * HK + h + 1],
                        ps_gx[:], OP.mult, OP.add,
                    )

                    t1 = apool.tile([P, BL], f32, tag="t1", bufs=2, name=f"t1_{h}")
                    if first:
                        nc.vector.tensor_scalar(
                            t1[:], r_t[:], bhhn_sb[:, h : h + 1], None, OP.mult
                        )
                    else:
                        whh_sb = wpool.tile(
                            [P, HK, P], WDT, tag="whh", bufs=10, name=f"whn{l}_{h}"
                        )
                        nc.sync.dma_start(whh_sb[:], whh_d.ap()[l, 2 * HK + h])
                        ps_gh = pspool.tile([P, BL], f32, tag="ps", name=f"ph{l}_{h}")
                        for k in range(HK):
                            nc.tensor.matmul(
                                ps_gh[:], whh_sb[:, k, :], zb_cur[k][:],
                                start=(k == 0), stop=(k == HK - 1),
                            )
                        nc.vector.scalar_tensor_tensor(
                            t1[:], ps_gh[:], bhhn_sb[:, h : h + 1], r_t[:],
                            OP.add, OP.mult,
                        )
                    npre = apool.tile([P, BL], f32, tag="npre", bufs=2, name=f"np{h}")
                    nc.vector.scalar_tensor_tensor(
                        npre[:], pre_n[:], gb_sb[:, 2 * HK + h : 2 * HK + h + 1],
                        t1[:], OP.add, OP.add,
                    )
                    n_t = apool.tile([P, BL], f32, tag="n", bufs=2, name=f"n{l}_{h}")
                    nc.scalar.activation(n_t[:], npre[:], AF.Tanh)
                    # z' = (n + STD*eps) + u*(z - n)
                    eps_sb = apool.tile([P, BL], f32, tag="eps", bufs=4, name=f"e{h}")
                    nc.sync.dma_start(eps_sb[:], epsT_d.ap()[l, :, h, :])
                    e1 = apool.tile([P, BL], f32, tag="e1", bufs=2, name=f"e1_{h}")
                    nc.vector.scalar_tensor_tensor(
                        e1[:], eps_sb[:], STD, n_t[:], OP.mult, OP.add
                    )
                    d_t = apool.tile([P, BL], f32, tag="d", bufs=2, name=f"d{h}")
                    nc.vector.tensor_sub(d_t[:], z_cur[h][:], n_t[:])
                    ud_t = apool.tile([P, BL], f32, tag="ud", bufs=2, name=f"ud{h}")
                    nc.vector.tensor_mul(ud_t[:], u_t[:], d_t[:])
                    zbt = apool.tile([P, BL], WDT, tag=f"zb{h}", bufs=2, name=f"zb{h}")
                    nc.vector.tensor_add(zbt[:], e1[:], ud_t[:])
                    zb_new.append(zbt)
                    zt = apool.tile([P, BL], f32, tag=f"z{h}", bufs=2, name=f"z{h}")
                    nc.vector.tensor_add(zt[:], e1[:], ud_t[:])
                    z_new.append(zt)

                z_cur, zb_cur = z_new, zb_new

                # ---- predict head -> t_{l+1} ----
                if not last:
                    w1_tiles = {}

                    def w1s(m, k, l=l, w1_tiles=w1_tiles):
                        if m not in w1_tiles:
                            w1_tiles[m] = wpool.tile(
                                [P, HK, P], WDT, tag="w1", bufs=8, name=f"w1_{l}_{m}"
                            )
                            nc.sync.dma_start(w1_tiles[m][:], w1_d.ap()[l, m])
                        return w1_tiles[m][:, k, :]

                    ps2 = predict(
                        l, w1s,
                        lambda m: w2_sb[:, m : m + 1],
                        lambda m: b1s_sb[:, m : m + 1],
                        [zb[:] for zb in zb_cur],
                    )
                    tmp_pending = (head_t(l, ps2), l + 1)

            # ---- final projections ----
            ps_mu = pspool.tile([Z, BL], f32, tag="ps", name="ps_mu")
            for k in range(HK):
                nc.tensor.matmul(
                    ps_mu[:], muW_sb[:, k, :], zb_cur[k][:],
                    start=(k == 0), stop=(k == HK - 1),
                )
            mu_sb = cpool.tile([Z, BL], f32)
            nc.scalar.activation(mu_sb[:], ps_mu[:], AF.Identity, bias=mub_sb[:, 0:1])
            ps_lv = pspool.tile([Z, BL], f32, tag="ps", name="ps_lv")
            for k in range(HK):
                nc.tensor.matmul(
                    ps_lv[:], lvW_sb[:, k, :], zb_cur[k][:],
                    start=(k == 0), stop=(k == HK - 1),
                )
            # softplus(x) = ln(exp(x) + 1): Exp+Ln live in one act table
            s1_sb = cpool.tile([Z, BL], f32)
            nc.scalar.activation(s1_sb[:], ps_lv[:], AF.Exp, bias=lvb_sb[:, 0:1])
            lv_sb = cpool.tile([Z, BL], f32)
            nc.scalar.activation(lv_sb[:], s1_sb[:], AF.Ln, bias=1.0)
            e_sb = cpool.tile([Z, BL], f32)
            nc.scalar.activation(e_sb[:], lv_sb[:], AF.Exp, bias=0.0, scale=0.5)
            prod_sb = cpool.tile([Z, BL], f32)
            nc.vector.tensor_mul(prod_sb[:], e_sb[:], epsf_sb[:])
            lat_sb = cpool.tile([Z, BL], f32)
            nc.vector.tensor_add(lat_sb[:], mu_sb[:], prod_sb[:])

            nc.sync.dma_start(lat_o.ap(), lat_sb[:])
            nc.sync.dma_start(mu_o.ap(), mu_sb[:])
            nc.sync.dma_start(lv_o.ap(), lv_sb[:])


def kernel(**inputs):
    from concourse import bacc
    from concourse.bass_utils import run_bass_kernel_spmd

    in_maps, pr_b2, fc_b2 = _pack_host(inputs)

    nc = bacc.Bacc("TRN2", target_bir_lowering=False, debug=False,
                   enable_asserts=False)
    _build(nc)
    nc.compile()

    res = run_bass_kernel_spmd(nc, in_maps, core_ids=list(range(NCORES)))

    latent = np.empty((B, Z), np.float32)
    mu_f = np.empty((B, Z), np.float32)
    log_var = np.empty((B, Z), np.float32)
    t = np.empty((B, L), np.float32)
    for c in range(NCORES):
        sl = slice(c * BL, (c + 1) * BL)
        latent[sl] = res.results[c]["lat_o"].T
        mu_f[sl] = res.results[c]["mu_o"].T
        log_var[sl] = res.results[c]["lv_o"].T
        t[sl] = res.results[c]["t_o"].T
    return latent, mu_f, log_var, t


# revision 38
# speedup vs baseline: 1.1973x; 1.1973x over previous
"""Trainium2 Bass kernel for nn_Encoder_82411832475895.

16-step GRU encoder with per-step weights + per-step Predict heads, then
mu/log_var projections.  Data-parallel over batch across 8 NeuronCores
(512 rows/core); activations kept transposed on-chip ([feature, batch])
so batch is the matmul free dim and weights are PE-stationary.

Self-contained: takes the full unsharded inputs, shards/packs on host,
compiles+runs the Bass kernel SPMD on cores 0-7, gathers to full output.
"""

import math

import numpy as np

B, X, H, Z, L = 4096, 512, 1024, 128, 16
NCORES = 8
BL = B // NCORES  # 512 batch rows per core
P = 128
HK = H // P  # 8
XK = X // P  # 4
GK = 3 * H // P  # 24 gate chunks: 0..7 r, 8..15 u(z), 16..23 n
STD = float(math.exp(0.5 * 0.005))

# "f32" (exact, 4 cyc/row) or "bf16"
COMPUTE_DT = "bf16"


def _pack_host(inputs):
    """Shard + lay out inputs for the on-chip program."""
    import ml_dtypes

    f32 = np.float32
    wnp = ml_dtypes.bfloat16 if COMPUTE_DT == "bf16" else f32

    g = {k: np.asarray(v, f32) for k, v in inputs.items()}

    x = g["x"]  # [B, X]
    eps_steps = g["eps_steps"]  # [L, B, H]
    eps_final = g["eps_final"]  # [B, Z]

    # lhsT chunk layout [.., m, i(K), k, j(M)]: w[m, i, k, j] = W[m*P+j, k*P+i]
    wih = np.ascontiguousarray(
        g["gru_Wih"][:, :, :X].reshape(L, GK, P, XK, P).transpose(0, 1, 4, 3, 2)
    ).astype(wnp)
    whh = np.ascontiguousarray(
        g["gru_Whh"].reshape(L, GK, P, HK, P).transpose(0, 1, 4, 3, 2)
    ).astype(wnp)
    w1 = np.ascontiguousarray(
        g["pr_W1"].reshape(L, HK, P, HK, P).transpose(0, 1, 4, 3, 2)
    ).astype(wnp)
    w2 = np.ascontiguousarray(
        g["pr_W2"][:, 0, :].reshape(L, HK, P).transpose(0, 2, 1)
    ).astype(wnp)  # [L, P(i), HK(k)]
    # wt: Wih t-column per gate chunk -> per-partition scalar [L, P, GK]
    wt = np.ascontiguousarray(
        g["gru_Wih"][:, :, X].reshape(L, GK, P).transpose(0, 2, 1)
    ).astype(f32)
    # gb: gate bias per chunk: bih+bhh for r/u, bih for n  [L, P, GK]
    gb = (g["gru_bih"] + g["gru_bhh"]).reshape(L, GK, P).copy()
    gb[:, 2 * HK :] = g["gru_bih"].reshape(L, GK, P)[:, 2 * HK :]
    gb = np.ascontiguousarray(gb.transpose(0, 2, 1)).astype(f32)
    # bhhn: recurrent bias of the n gate [L, P, HK]
    bhhn = np.ascontiguousarray(
        g["gru_bhh"][:, 2 * H :].reshape(L, HK, P).transpose(0, 2, 1)
    ).astype(f32)
    b1s = np.ascontiguousarray(
        g["pr_b1"].reshape(L, HK, P).transpose(0, 2, 1)
    ).astype(f32)
    fw1 = np.ascontiguousarray(
        g["fc_W1"].reshape(HK, P, XK, P).transpose(3, 0, 2, 1)
    ).astype(wnp)  # [P(i), HK(m), XK(k), P(j)]
    fb1 = np.ascontiguousarray(g["fc_b1"].reshape(HK, P).T).astype(f32)
    fw2 = np.ascontiguousarray(g["fc_W2"][0].reshape(HK, P).T).astype(wnp)
    muW = np.ascontiguousarray(
        g["mu_W"].T.reshape(HK, P, Z).transpose(1, 0, 2)
    ).astype(wnp)  # [P(i), HK(k), Z(j)]
    lvW = np.ascontiguousarray(
        g["lv_W"].T.reshape(HK, P, Z).transpose(1, 0, 2)
    ).astype(wnp)
    mub = np.ascontiguousarray(g["mu_b"].reshape(Z, 1)).astype(f32)
    lvb = np.ascontiguousarray(g["lv_b"].reshape(Z, 1)).astype(f32)

    pr_b2 = [float(v) for v in np.asarray(g["pr_b2"]).reshape(L)]
    fc_b2 = float(np.asarray(g["fc_b2"]).reshape(()))
    b2s = np.array([[fc_b2] + pr_b2], f32).astype(wnp)  # [1, L+1]

    shared = {
        "wih": wih, "whh": whh, "w1": w1, "w2": w2, "wt": wt, "gb": gb,
        "bhhn": bhhn, "b1s": b1s, "fw1": fw1, "fb1": fb1, "fw2": fw2,
        "muW": muW, "lvW": lvW, "mub": mub, "lvb": lvb, "b2s": b2s,
    }

    in_maps = []
    for c in range(NCORES):
        sl = slice(c * BL, (c + 1) * BL)
        xT = np.ascontiguousarray(
            x[sl].T.reshape(XK, P, BL).transpose(1, 0, 2)
        ).astype(wnp)  # [P(i), XK(k), BL]
        epsT = np.ascontiguousarray(
            eps_steps[:, sl, :].transpose(0, 2, 1).reshape(L, HK, P, BL)
            .transpose(0, 2, 1, 3)
        ).astype(f32)  # [L, P(i), HK(h), BL]
        epsf = np.ascontiguousarray(eps_final[sl].T).astype(f32)  # [Z, BL]
        in_maps.append({"xT": xT, "epsT": epsT, "epsf": epsf, **shared})

    return in_maps, pr_b2, fc_b2


def _build(nc):
    import concourse.bass as bass  # noqa: F401
    import concourse.tile as tile
    from concourse import mybir
    from concourse.alu_op_type import AluOpType as OP

    AF = mybir.ActivationFunctionType
    f32 = mybir.dt.float32
    WDT = mybir.dt.bfloat16 if COMPUTE_DT == "bf16" else f32

    # ---- DRAM tensors ----
    xT_d = nc.dram_tensor("xT", [P, XK, BL], WDT, kind="ExternalInput")
    epsT_d = nc.dram_tensor("epsT", [L, P, HK, BL], f32, kind="ExternalInput")
    epsf_d = nc.dram_tensor("epsf", [Z, BL], f32, kind="ExternalInput")
    wih_d = nc.dram_tensor("wih", [L, GK, P, XK, P], WDT, kind="ExternalInput")
    whh_d = nc.dram_tensor("whh", [L, GK, P, HK, P], WDT, kind="ExternalInput")
    w1_d = nc.dram_tensor("w1", [L, HK, P, HK, P], WDT, kind="ExternalInput")
    w2_d = nc.dram_tensor("w2", [L, P, HK], WDT, kind="ExternalInput")
    wt_d = nc.dram_tensor("wt", [L, P, GK], f32, kind="ExternalInput")
    gb_d = nc.dram_tensor("gb", [L, P, GK], f32, kind="ExternalInput")
    bhhn_d = nc.dram_tensor("bhhn", [L, P, HK], f32, kind="ExternalInput")
    b1s_d = nc.dram_tensor("b1s", [L, P, HK], f32, kind="ExternalInput")
    fw1_d = nc.dram_tensor("fw1", [P, HK, XK, P], WDT, kind="ExternalInput")
    fb1_d = nc.dram_tensor("fb1", [P, HK], f32, kind="ExternalInput")
    fw2_d = nc.dram_tensor("fw2", [P, HK], WDT, kind="ExternalInput")
    muW_d = nc.dram_tensor("muW", [P, HK, Z], WDT, kind="ExternalInput")
    lvW_d = nc.dram_tensor("lvW", [P, HK, Z], WDT, kind="ExternalInput")
    mub_d = nc.dram_tensor("mub", [Z, 1], f32, kind="ExternalInput")
    lvb_d = nc.dram_tensor("lvb", [Z, 1], f32, kind="ExternalInput")
    b2s_d = nc.dram_tensor("b2s", [1, L + 1], WDT, kind="ExternalInput")
    lat_o = nc.dram_tensor("lat_o", [Z, BL], f32, kind="ExternalOutput")
    mu_o = nc.dram_tensor("mu_o", [Z, BL], f32, kind="ExternalOutput")
    lv_o = nc.dram_tensor("lv_o", [Z, BL], f32, kind="ExternalOutput")
    t_o = nc.dram_tensor("t_o", [L, BL], f32, kind="ExternalOutput")

    with tile.TileContext(nc) as tc:
        with (
            tc.tile_pool(name="const", bufs=1) as cpool,
            tc.tile_pool(name="wts", bufs=1) as wpool,
            tc.tile_pool(name="act", bufs=1) as apool,
            tc.tile_pool(name="psum", bufs=8, space="PSUM") as pspool,
        ):
            # ---- resident constants ----
            x_sb = cpool.tile([P, XK, BL], WDT)
            for k in range(XK):
                nc.sync.dma_start(x_sb[:, k, :], xT_d.ap()[:, k, :])
            epsf_sb = cpool.tile([Z, BL], f32)
            nc.sync.dma_start(epsf_sb[:], epsf_d.ap())
            fb1_sb = cpool.tile([P, HK], f32)
            nc.sync.dma_start(fb1_sb[:], fb1_d.ap())
            fw2_sb = cpool.tile([P, HK], WDT)
            nc.sync.dma_start(fw2_sb[:], fw2_d.ap())
            muW_sb = cpool.tile([P, HK, Z], WDT)
            nc.sync.dma_start(muW_sb[:], muW_d.ap())
            lvW_sb = cpool.tile([P, HK, Z], WDT)
            nc.sync.dma_start(lvW_sb[:], lvW_d.ap())
            mub_sb = cpool.tile([Z, 1], f32)
            nc.sync.dma_start(mub_sb[:], mub_d.ap())
            lvb_sb = cpool.tile([Z, 1], f32)
            nc.sync.dma_start(lvb_sb[:], lvb_d.ap())
            fw1_sb = cpool.tile([P, HK, XK, P], WDT)
            for m in range(HK):
                nc.sync.dma_start(fw1_sb[:, m], fw1_d.ap()[:, m])
            b2_sb = cpool.tile([1, L + 1], WDT)
            nc.sync.dma_start(b2_sb[:], b2s_d.ap())
            ones_sb = cpool.tile([1, BL], WDT)
            nc.vector.memset(ones_sb[:], 1.0)
            ones128 = cpool.tile([1, P], WDT)
            nc.vector.memset(ones128[:], 1.0)

            # per-h z state tiles (separate tiles -> fine-grained deps)
            z_cur = []
            zb_cur = []
            for h in range(HK):
                zt = apool.tile([P, BL], f32, tag=f"z{h}", bufs=2, name=f"z0_{h}")
                nc.vector.memset(zt[:], 0.0)
                z_cur.append(zt)
                zbt = apool.tile([P, BL], WDT, tag=f"zb{h}", bufs=2, name=f"zb0_{h}")
                nc.vector.memset(zbt[:], 0.0)
                zb_cur.append(zbt)

            def predict(l, w1s, w2s, b1ss, zb, is_first_cell=False):
                """Two-wave W1/W2 head; returns the logit psum [1, BL]."""
                ps2 = pspool.tile([1, BL], f32, tag="ps", name=f"ps2_{l}")
                psp = {}
                for wave in (range(0, 4), range(4, 8)):
                    for m in wave:
                        psp[m] = pspool.tile([P, BL], f32, tag="ps", name=f"pp{l}_{m}")
                        for k in range(HK - 1):
                            nc.tensor.matmul(
                                psp[m][:], w1s(m, k), zb[k], start=(k == 0), stop=False
                            )
                    for m in wave:
                        nc.tensor.matmul(
                            psp[m][:], w1s(m, HK - 1), zb[HK - 1],
                            start=False, stop=True,
                        )
                    for m in wave:
                        h1 = apool.tile([P, BL], WDT, tag="h1", bufs=4, name=f"h1_{m}")
                        nc.scalar.activation(h1[:], psp[m][:], AF.Relu, bias=b1ss(m))
                        nc.tensor.matmul(
                            ps2[:], w2s(m), h1[:], start=(m == 0), stop=False
                        )
                bidx = 0 if is_first_cell else l + 1
                nc.tensor.matmul(
                    ps2[:], b2_sb[0:1, bidx : bidx + 1], ones_sb[:],
                    start=False, stop=True,
                )
                return ps2

            def head_t(l, ps2):
                """sigmoid -> t row DMA; is_ge -> tmp row (partition 0)."""
                row = 0 if l is None else l + 1
                t_row = apool.tile([1, BL], f32, tag="trow", bufs=2, name=f"tr{row}")
                nc.scalar.activation(t_row[:], ps2[:], AF.Sigmoid)
                nc.sync.dma_start(t_o.ap()[row : row + 1, :], t_row[:])
                tmp_row = apool.tile([1, BL], WDT, tag="tmp", bufs=2, name=f"tm{row}")
                nc.vector.tensor_scalar(tmp_row[:], ps2[:], 0.0, None, OP.is_ge)
                return tmp_row

            def bcast_tmp(tmp_row, row):
                """Broadcast tmp to all partitions: rank-1 matmul + copy.
                Emitted a little into the next step so the PE FIFO isn't
                head-of-line blocked waiting for is_ge."""
                ps_b = pspool.tile([P, BL], f32, tag="ps", name=f"psb{row}")
                nc.tensor.matmul(ps_b[:], ones128[:], tmp_row[:], start=True, stop=True)
                tmp_bc = apool.tile([P, BL], f32, tag="tbc", bufs=2, name=f"tb{row}")
                nc.scalar.copy(tmp_bc[:], ps_b[:])
                return tmp_bc

            # ---- firstcell (K=XK, written out explicitly) ----
            ps2 = pspool.tile([1, BL], f32, tag="ps", name="ps2_fc")
            for m in range(HK):
                psf = pspool.tile([P, BL], f32, tag="ps", name="ps_fc")
                for k in range(XK):
                    nc.tensor.matmul(
                        psf[:], fw1_sb[:, m, k, :], x_sb[:, k, :],
                        start=(k == 0), stop=(k == XK - 1),
                    )
                h1 = apool.tile([P, BL], WDT, tag="h1", bufs=4, name="h1_fc")
                nc.scalar.activation(h1[:], psf[:], AF.Relu, bias=fb1_sb[:, m : m + 1])
                nc.tensor.matmul(
                    ps2[:], fw2_sb[:, m : m + 1], h1[:], start=(m == 0), stop=False
                )
            nc.tensor.matmul(
                ps2[:], b2_sb[0:1, 0:1], ones_sb[:], start=False, stop=True
            )
            tmp_pending = (head_t(None, ps2), 0)
            tmp_bc = None

            # ---- L GRU steps ----
            for l in range(L):
                first, last = l == 0, l == L - 1
                wt_sb = wpool.tile([P, GK], f32, tag="wt", bufs=2, name=f"wt{l}")
                nc.sync.dma_start(wt_sb[:], wt_d.ap()[l])
                gb_sb = wpool.tile([P, GK], f32, tag="gb", bufs=2, name=f"gb{l}")
                nc.sync.dma_start(gb_sb[:], gb_d.ap()[l])
                bhhn_sb = wpool.tile([P, HK], f32, tag="bhhn", bufs=2, name=f"bh{l}")
                nc.sync.dma_start(bhhn_sb[:], bhhn_d.ap()[l])
                if not last:
                    b1s_sb = wpool.tile([P, HK], f32, tag="b1s", bufs=2, name=f"b1{l}")
                    nc.sync.dma_start(b1s_sb[:], b1s_d.ap()[l])
                    w2_sb = wpool.tile([P, HK], WDT, tag="w2", bufs=2, name=f"w2{l}")
                    nc.sync.dma_start(w2_sb[:], w2_d.ap()[l])

                z_new, zb_new = [], []

                def gate_mms(m, with_z, l=l, first=first):
                    """PSUM accumulation of gate chunk m (x part [+ z part])."""
                    wih_sb = wpool.tile(
                        [P, XK, P], WDT, tag="wih", bufs=12, name=f"wi{l}_{m}"
                    )
                    nc.sync.dma_start(wih_sb[:], wih_d.ap()[l, m])
                    ps = pspool.tile([P, BL], f32, tag="ps", name=f"ps{l}_{m}")
                    use_z = with_z and not first
                    for k in range(XK):
                        nc.tensor.matmul(
                            ps[:], wih_sb[:, k, :], x_sb[:, k, :],
                            start=(k == 0), stop=(k == XK - 1 and not use_z),
                        )
                    if use_z:
                        whh_sb = wpool.tile(
                            [P, HK, P], WDT, tag="whh", bufs=10, name=f"wh{l}_{m}"
                        )
                        nc.sync.dma_start(whh_sb[:], whh_d.ap()[l, m])
                        for k in range(HK):
                            nc.tensor.matmul(
                                ps[:], whh_sb[:, k, :], zb_cur[k][:],
                                start=False, stop=(k == HK - 1),
                            )
                    return ps

                for h in range(HK):
                    ps_r = gate_mms(h, True)
                    if h == 0:
                        tmp_bc = bcast_tmp(*tmp_pending)
                    pre_r = apool.tile([P, BL], f32, tag="pre", bufs=3, name=f"pr{h}")
                    nc.vector.scalar_tensor_tensor(
                        pre_r[:], tmp_bc[:], wt_sb[:, h : h + 1], ps_r[:],
                        OP.mult, OP.add,
                    )
                    r_t = apool.tile([P, BL], f32, tag="r", bufs=2, name=f"r{l}_{h}")
                    nc.scalar.activation(
                        r_t[:], pre_r[:], AF.Sigmoid, bias=gb_sb[:, h : h + 1]
                    )

                    ps_u = gate_mms(HK + h, True)
                    pre_u = apool.tile([P, BL], f32, tag="pre", bufs=3, name=f"pu{h}")
                    nc.vector.scalar_tensor_tensor(
                        pre_u[:], tmp_bc[:], wt_sb[:, HK + h : HK + h + 1], ps_u[:],
                        OP.mult, OP.add,
                    )
                    u_t = apool.tile([P, BL], f32, tag="u", bufs=2, name=f"u{l}_{h}")
                    nc.scalar.activation(
                        u_t[:], pre_u[:], AF.Sigmoid, bias=gb_sb[:, HK + h : HK + h + 1]
                    )

                    ps_gx = gate_mms(2 * HK + h, False)
                    pre_n = apool.tile([P, BL], f32, tag="pre", bufs=3, name=f"pn{h}")
                    nc.vector.scalar_tensor_tensor(
                        pre_n[:], tmp_bc[:], wt_sb[:, 2 * HK + h : 2 * HK + h + 1],
                        ps_gx[:], OP.mult, OP.add,
                    )

                    t1 = apool.tile([P, BL], f32, tag="t1", bufs=2, name=f"t1_{h}")
                    if first:
                        nc.vector.tensor_scalar(
                            t1[:], r_t[:], bhhn_sb[:, h : h + 1], None, OP.mult
                        )
                    else:
                        whh_sb = wpool.tile(
                            [P, HK, P], WDT, tag="whh", bufs=10, name=f"whn{l}_{h}"
                        )
                        nc.sync.dma_start(whh_sb[:], whh_d.ap()[l, 2 * HK + h])
                        ps_gh = pspool.tile([P, BL], f32, tag="ps", name=f"ph{l}_{h}")
                        for k in range(HK):
                            nc.tensor.matmul(
                                ps_gh[:], whh_sb[:, k, :], zb_cur[k][:],
                                start=(k == 0), stop=(k == HK - 1),
                            )
                        nc.vector.scalar_tensor_tensor(
                            t1[:], ps_gh[:], bhhn_sb[:, h : h + 1], r_t[:],
                            OP.add, OP.mult,
                        )
                    npre = apool.tile([P, BL], f32, tag="npre", bufs=2, name=f"np{h}")
                    nc.vector.scalar_tensor_tensor(
                        npre[:], pre_n[:], gb_sb[:, 2 * HK + h : 2 * HK + h + 1],
                        t1[:], OP.add, OP.add,
                    )
                    n_t = apool.tile([P, BL], f32, tag="n", bufs=2, name=f"n{l}_{h}")
                    nc.scalar.activation(n_t[:], npre[:], AF.Tanh)
                    # z' = (n + u*(z - n)) + STD*eps
                    eps_sb = apool.tile([P, BL], f32, tag="eps", bufs=4, name=f"e{h}")
                    nc.sync.dma_start(eps_sb[:], epsT_d.ap()[l, :, h, :])
                    d_t = apool.tile([P, BL], f32, tag="d", bufs=2, name=f"d{h}")
                    nc.vector.tensor_sub(d_t[:], z_cur[h][:], n_t[:])
                    ud_t = apool.tile([P, BL], f32, tag="ud", bufs=2, name=f"ud{h}")
                    nc.vector.tensor_mul(ud_t[:], u_t[:], d_t[:])
                    mu_t = apool.tile([P, BL], f32, tag="mu", bufs=2, name=f"m{h}")
                    nc.vector.tensor_add(mu_t[:], n_t[:], ud_t[:])
                    zt = apool.tile([P, BL], f32, tag=f"z{h}", bufs=2, name=f"z{h}")
                    nc.vector.scalar_tensor_tensor(
                        zt[:], eps_sb[:], STD, mu_t[:], OP.mult, OP.add
                    )
                    z_new.append(zt)
                    zbt = apool.tile([P, BL], WDT, tag=f"zb{h}", bufs=2, name=f"zb{h}")
                    nc.scalar.copy(zbt[:], zt[:])
                    zb_new.append(zbt)

                z_cur, zb_cur = z_new, zb_new

                # ---- predict head -> t_{l+1} ----
                if not last:
                    w1_tiles = {}

                    def w1s(m, k, l=l, w1_tiles=w1_tiles):
                        if m not in w1_tiles:
                            w1_tiles[m] = wpool.tile(
                                [P, HK, P], WDT, tag="w1", bufs=8, name=f"w1_{l}_{m}"
                            )
                            nc.sync.dma_start(w1_tiles[m][:], w1_d.ap()[l, m])
                        return w1_tiles[m][:, k, :]

                    ps2 = predict(
                        l, w1s,
                        lambda m: w2_sb[:, m : m + 1],
                        lambda m: b1s_sb[:, m : m + 1],
                        [zb[:] for zb in zb_cur],
                    )
                    tmp_pending = (head_t(l, ps2), l + 1)

            # ---- final projections ----
            ps_mu = pspool.tile([Z, BL], f32, tag="ps", name="ps_mu")
            for k in range(HK):
                nc.tensor.matmul(
                    ps_mu[:], muW_sb[:, k, :], zb_cur[k][:],
                    start=(k == 0), stop=(k == HK - 1),
                )
            mu_sb = cpool.tile([Z, BL], f32)
            nc.scalar.activation(mu_sb[:], ps_mu[:], AF.Identity, bias=mub_sb[:, 0:1])
            ps_lv = pspool.tile([Z, BL], f32, tag="ps", name="ps_lv")
            for k in range(HK):
                nc.tensor.matmul(
                    ps_lv[:], lvW_sb[:, k, :], zb_cur[k][:],
                    start=(k == 0), stop=(k == HK - 1),
                )
            # softplus(x) = ln(exp(x) + 1): Exp+Ln live in one act table
            s1_sb = cpool.tile([Z, BL], f32)
            nc.scalar.activation(s1_sb[:], ps_lv[:], AF.Exp, bias=lvb_sb[:, 0:1])
            lv_sb = cpool.tile([Z, BL], f32)
            nc.scalar.activation(lv_sb[:], s1_sb[:], AF.Ln, bias=1.0)
            e_sb = cpool.tile([Z, BL], f32)
            nc.scalar.activation(e_sb[:], lv_sb[:], AF.Exp, bias=0.0, scale=0.5)
            prod_sb = cpool.tile([Z, BL], f32)
            nc.vector.tensor_mul(prod_sb[:], e_sb[:], epsf_sb[:])
            lat_sb = cpool.tile([Z, BL], f32)
            nc.vector.tensor_add(lat_sb[:], mu_sb[:], prod_sb[:])

            nc.sync.dma_start(lat_o.ap(), lat_sb[:])
            nc.sync.dma_start(mu_o.ap(), mu_sb[:])
            nc.sync.dma_start(lv_o.ap(), lv_sb[:])


def kernel(**inputs):
    from concourse import bacc
    from concourse.bass_utils import run_bass_kernel_spmd

    in_maps, pr_b2, fc_b2 = _pack_host(inputs)

    nc = bacc.Bacc("TRN2", target_bir_lowering=False, debug=False,
                   enable_asserts=False)
    _build(nc)
    nc.compile()

    res = run_bass_kernel_spmd(nc, in_maps, core_ids=list(range(NCORES)))

    latent = np.empty((B, Z), np.float32)
    mu_f = np.empty((B, Z), np.float32)
    log_var = np.empty((B, Z), np.float32)
    t = np.empty((B, L), np.float32)
    for c in range(NCORES):
        sl = slice(c * BL, (c + 1) * BL)
        latent[sl] = res.results[c]["lat_o"].T
        mu_f[sl] = res.results[c]["mu_o"].T
        log_var[sl] = res.results[c]["lv_o"].T
        t[sl] = res.results[c]["t_o"].T
    return latent, mu_f, log_var, t
